# revision 34
# baseline (speedup 1.0000x reference)
"""AngularPhysicsGate distributed Trainium2 kernel (8 NeuronCores).

reference (per batch b):
    x_unit = x / clip(||x||_row, eps)
    K      = x_unit @ x_unit^T                      [L, L]
    K_norm = (K - mean(K)) / clip(std(K, ddof=1), eps)
    A      = mean_h attn[b]                         [L, L]
    cos    = sum_m A * K_norm                       [L]
    gate   = sigmoid(W * cos + b)

Sharding: 8 cores = 2 batches x 4 row-chunks of 512 rows.  Each core:
  - loads the full x[b] (rolled so its own 512 rows sit at rows 0..511,
    keeping the SPMD program identical across cores), normalizes rows,
    transposes to [d, L] bf16 via the PE array,
  - computes its K rows with PE matmuls (bf16 operands, fp32 PSUM accum),
  - accumulates sum / sum-of-squares of K, AllReduces them across the 4
    cores of its batch to get the global mean/std,
  - streams its attn rows, head-sums on DVE, and folds cos_align through
    the raw-K linear decomposition:
        cos = inv_std * (sum_m A*K - mean * sum_m A)
  - writes K_norm rows, cos, gate.
Host side un-rolls the column rotation and concatenates shards.
"""

import numpy as np

import concourse.bass as bass
import concourse.bacc as bacc
import concourse.tile as tile
import concourse.mybir as mybir
from concourse import bass_isa
from concourse.masks import make_identity
from concourse.bass_utils import run_bass_kernel_spmd

F32 = mybir.dt.float32
F32R = mybir.dt.float32r
BF16 = mybir.dt.bfloat16
ALU = mybir.AluOpType
ACTF = mybir.ActivationFunctionType

B, L, D, H = 2, 2048, 2048, 8
P = 128
RC = L // 4                 # 512 rows per core
NSLAB = L // P              # 16 x-slabs
NK = D // P                 # 16 contraction tiles
NR = RC // P                # 4 row subtiles per core
NC_FREE = 512               # matmul moving free dim / PSUM bank
NCG = L // NC_FREE          # 4 column groups
N_CORES = 8
EPS = 1e-6
NTOT = float(L) * float(L)

REPLICA_GROUPS = [[0, 1, 2, 3], [4, 5, 6, 7]]

_NC_CACHE = None


def _build_kernel():
    global _NC_CACHE
    if _NC_CACHE is not None:
        return _NC_CACHE

    nc = bacc.Bacc(
        "TRN2",
        target_bir_lowering=False,
        debug=False,
        num_devices=N_CORES,
    )

    x_ext = nc.dram_tensor("x", [L, D], F32, kind="ExternalInput")
    at_ext = nc.dram_tensor("attn", [H, RC, L], F32, kind="ExternalInput")
    wb_ext = nc.dram_tensor("wb", [1, 2], F32, kind="ExternalInput")
    kn_ext = nc.dram_tensor("k_norm", [RC, L], F32, kind="ExternalOutput")
    cos_ext = nc.dram_tensor("cos", [P, NR], F32, kind="ExternalOutput")
    gate_ext = nc.dram_tensor("gate", [P, NR], F32, kind="ExternalOutput")

    with tile.TileContext(nc) as tc:
        _body(nc, tc, x_ext, at_ext, wb_ext, kn_ext, cos_ext, gate_ext)

    nc.compile()
    _NC_CACHE = nc
    return nc


def _body(nc, tc, x_ext, at_ext, wb_ext, kn_ext, cos_ext, gate_ext):
    with (
        tc.tile_pool(name="consts", bufs=1) as consts,
        tc.tile_pool(name="xin", bufs=3) as xin_pool,
        tc.tile_pool(name="xu", bufs=2) as xu_pool,
        tc.tile_pool(name="xut", bufs=1) as xut_pool,
        tc.tile_pool(name="ksb", bufs=1) as ksb_pool,
        tc.tile_pool(name="attn", bufs=5) as at_pool,
        tc.tile_pool(name="acc", bufs=2) as acc_pool,
        tc.tile_pool(name="scr", bufs=2) as scr_pool,
        tc.tile_pool(name="small", bufs=1) as small,
        tc.tile_pool(name="psmm", bufs=4, space="PSUM") as psmm_pool,
        tc.tile_pool(name="pstr", bufs=3, space="PSUM") as pstr_pool,
        tc.tile_pool(name="dram", bufs=1, space="DRAM") as dram_pool,
    ):
        ident = consts.tile([P, P], BF16)
        make_identity(nc, ident)

        # W / bias, broadcast to all partitions
        wb_sb = small.tile([1, 2], F32)
        nc.sync.dma_start(wb_sb[:], wb_ext[:, :])
        wb_bc = small.tile([P, 2], F32)
        nc.gpsimd.partition_broadcast(wb_bc[:], wb_sb[:], channels=P)

        xuT = xut_pool.tile([P, NK, L], BF16)     # x_unit^T, [d-part, ktile, row]
        K_sb = ksb_pool.tile([P, NR, L], BF16)    # this core's K rows (cosine)

        ss_all = small.tile([P, NSLAB], F32)      # row sum-of-squares per slab
        inv_all = small.tile([P, NSLAB], F32)     # 1/max(sqrt(ss), eps)

        # ---- Stage 1: load x (scalar-engine queues), normalize rows,
        # cast bf16, PE-transpose to xuT ----
        last_x_dma = None
        for s in range(NSLAB):
            xs = xin_pool.tile([P, D], F32, tag="xs")
            last_x_dma = nc.sync.dma_start(xs[:], x_ext[s * P:(s + 1) * P, :])
            sq = scr_pool.tile([P, D], BF16, tag="sq")
            nc.scalar.activation(
                sq[:], xs[:], ACTF.Square, accum_out=ss_all[:, s:s + 1]
            )
            ssl = slice(s, s + 1)
            nc.scalar.activation(inv_all[:, ssl], ss_all[:, ssl], ACTF.Sqrt)
            nc.vector.tensor_scalar_max(inv_all[:, ssl], inv_all[:, ssl], EPS)
            nc.vector.reciprocal(inv_all[:, ssl], inv_all[:, ssl])
            xu = xu_pool.tile([P, D], BF16, tag="xu")
            nc.vector.tensor_scalar_mul(xu[:], xs[:], inv_all[:, ssl])
            for tg in range(4):
                ps = pstr_pool.tile([P, 4, P], BF16, tag="pstr")
                for t4 in range(4):
                    t = tg * 4 + t4
                    nc.tensor.transpose(
                        ps[:, t4], xu[:, t * P:(t + 1) * P], ident
                    )
                dst = xuT[:, tg * 4:(tg + 1) * 4, s * P:(s + 1) * P]
                if tg % 2 == 0:
                    nc.scalar.activation(dst, ps[:, :, :], ACTF.Copy)
                else:
                    nc.vector.tensor_copy(dst, ps[:, :, :])

        # ---- Stage 2: K = xu @ xu^T; drains carry the stats for free ----
        sum_slots = small.tile([P, NR * NCG], F32)
        ssq_slots = small.tile([P, NR * NCG], F32)
        for c in range(NCG):
            for r in range(NR):
                pk = psmm_pool.tile([P, NC_FREE], F32, tag="pk")
                for k in range(NK):
                    nc.tensor.matmul(
                        pk[:],
                        xuT[:, k, r * P:(r + 1) * P],
                        xuT[:, k, c * NC_FREE:(c + 1) * NC_FREE],
                        start=(k == 0),
                        stop=(k == NK - 1),
                    )
                idx = r * NCG + c
                csl = slice(c * NC_FREE, (c + 1) * NC_FREE)
                nc.scalar.activation(
                    K_sb[:, r, csl], pk[:], ACTF.Identity,
                    accum_out=sum_slots[:, idx:idx + 1],
                )
                sq2 = scr_pool.tile([P, NC_FREE], BF16, tag="sq2")
                nc.scalar.activation(
                    sq2[:], pk[:], ACTF.Square,
                    accum_out=ssq_slots[:, idx:idx + 1],
                )

        # ---- Stage 2b: global stats via AllReduce over the batch group ----
        stats2 = small.tile([P, 2], F32)
        nc.vector.tensor_reduce(
            stats2[:, 0:1], sum_slots[:], axis=mybir.AxisListType.X, op=ALU.add
        )
        nc.vector.tensor_reduce(
            stats2[:, 1:2], ssq_slots[:], axis=mybir.AxisListType.X, op=ALU.add
        )
        statsr = small.tile([P, 2], F32)
        nc.gpsimd.partition_all_reduce(
            statsr[:], stats2[:], channels=P, reduce_op=bass_isa.ReduceOp.add
        )
        stats_in = dram_pool.tile([P, 2], F32)
        stats_out = dram_pool.tile([P, 2], F32)
        nc.gpsimd.dma_start(stats_in[:], statsr[:])
        nc.gpsimd.collective_compute(
            "AllReduce",
            ALU.add,
            replica_groups=REPLICA_GROUPS,
            ins=[stats_in.opt()],
            outs=[stats_out.opt()],
        )
        statsg = small.tile([P, 2], F32)
        nc.gpsimd.dma_start(statsg[:], stats_out[:])

        # derived scalars, computed redundantly on every partition [P,1]
        mean_t = small.tile([P, 1], F32)
        nc.vector.tensor_scalar_mul(mean_t[:], statsg[:, 0:1], 1.0 / NTOT)
        t2 = small.tile([P, 1], F32)                              # S*mean - SS
        nc.vector.scalar_tensor_tensor(
            t2[:], statsg[:, 0:1], mean_t[:, 0:1], statsg[:, 1:2],
            op0=ALU.mult, op1=ALU.subtract,
        )
        std_t = small.tile([P, 1], F32)                           # sqrt(-t2/(N-1))
        nc.scalar.activation(std_t[:], t2[:], ACTF.Sqrt,
                             scale=-1.0 / (NTOT - 1.0))
        nc.vector.tensor_scalar_max(std_t[:], std_t[:], EPS)
        iv_t = small.tile([P, 1], F32)
        nc.vector.reciprocal(iv_t[:], std_t[:])
        s1_t = small.tile([P, 1], F32)                            # inv_std/H
        nc.vector.tensor_scalar_mul(s1_t[:], iv_t[:], 1.0 / H)
        s2_t = small.tile([P, 1], F32)                            # mean*inv_std/H
        nc.vector.tensor_mul(s2_t[:], mean_t[:], s1_t[:])
        nb_t = small.tile([P, 1], F32)                            # -mean*inv_std
        nc.vector.tensor_scalar_mul(nb_t[:], s2_t[:], -float(H))

        # ---- Stage 4: K_norm = K * inv_std - mean*inv_std, DMA out ----
        for r in range(NR):
            kn = xin_pool.tile([P, L], F32, tag="xs", name=f"kn_{r}")
            nc.scalar.activation(
                kn[:], K_sb[:, r, :], ACTF.Identity,
                bias=nb_t[:, 0:1], scale=iv_t[:, 0:1],
            )
            nc.scalar.dma_start(kn_ext[r * P:(r + 1) * P, :], kn[:])

        # ---- Stage 3: attn head-sum (DVE) + dot with cosine K ----
        dot_slots = small.tile([P, NR], F32)
        row_slots = small.tile([P, NR], F32)
        cos_sb = small.tile([P, NR], F32)
        gate_sb = small.tile([P, NR], F32)
        tmp = small.tile([P, NR], F32)
        for r in range(NR):
            a_acc = acc_pool.tile([P, L], F32, tag="acc")
            at0 = at_pool.tile([P, L], F32, tag="at")
            nc.sync.dma_start(at0[:], at_ext[0, r * P:(r + 1) * P, :])
            at1 = at_pool.tile([P, L], F32, tag="at")
            nc.sync.dma_start(at1[:], at_ext[1, r * P:(r + 1) * P, :])
            nc.vector.tensor_add(a_acc[:], at0[:], at1[:])
            for h in range(2, H):
                at = at_pool.tile([P, L], F32, tag="at")
                nc.sync.dma_start(at[:], at_ext[h, r * P:(r + 1) * P, :])
                if h < H - 1:
                    nc.vector.tensor_add(a_acc[:], a_acc[:], at[:])
                else:
                    # last head add also produces sum_m A (times H)
                    nc.vector.scalar_tensor_tensor(
                        a_acc[:], at[:], 1.0, a_acc[:],
                        op0=ALU.mult, op1=ALU.add,
                        accum_out=row_slots[:, r:r + 1],
                    )
            junk = scr_pool.tile([P, L], BF16, tag="sq")
            nc.vector.scalar_tensor_tensor(
                junk[:], a_acc[:], 1.0, K_sb[:, r, :],
                op0=ALU.mult, op1=ALU.mult,
                accum_out=dot_slots[:, r:r + 1],
            )
            # cos = s1 * dot - s2 * rowsum ; gate = sigmoid(W*cos + b)
            rsl = slice(r, r + 1)
            nc.vector.tensor_scalar_mul(tmp[:, rsl], row_slots[:, rsl],
                                        s2_t[:, 0:1])
            nc.vector.scalar_tensor_tensor(
                cos_sb[:, rsl], dot_slots[:, rsl], s1_t[:, 0:1], tmp[:, rsl],
                op0=ALU.mult, op1=ALU.subtract,
            )
            nc.scalar.activation(
                gate_sb[:, rsl], cos_sb[:, rsl], ACTF.Sigmoid,
                bias=wb_bc[:, 1:2], scale=wb_bc[:, 0:1],
            )
        nc.sync.dma_start(cos_ext[:, :], cos_sb[:])
        nc.sync.dma_start(gate_ext[:, :], gate_sb[:])


def _make_in_maps(x_spatial, attn, W, b):
    x_spatial = np.ascontiguousarray(x_spatial, dtype=np.float32)
    attn = np.ascontiguousarray(attn, dtype=np.float32)
    wb = np.array([[np.float32(np.ravel(W)[0]), np.float32(np.ravel(b)[0])]],
                  dtype=np.float32)
    in_maps = []
    for core in range(N_CORES):
        bb, rc = divmod(core, 4)
        r0 = rc * RC
        x_roll = np.roll(x_spatial[bb], -r0, axis=0)
        at_roll = np.roll(attn[bb, :, r0:r0 + RC, :], -r0, axis=2)
        in_maps.append({
            "x": np.ascontiguousarray(x_roll),
            "attn": np.ascontiguousarray(at_roll),
            "wb": wb,
        })
    return in_maps


def _assemble(results):
    gate = np.empty((B, L), np.float32)
    cos = np.empty((B, L), np.float32)
    K_norm = np.empty((B, L, L), np.float32)
    for core in range(N_CORES):
        bb, rc = divmod(core, 4)
        r0 = rc * RC
        out = results[core]
        K_norm[bb, r0:r0 + RC] = np.roll(out["k_norm"], r0, axis=1)
        cos[bb, r0:r0 + RC] = out["cos"].T.reshape(RC)
        gate[bb, r0:r0 + RC] = out["gate"].T.reshape(RC)
    return gate, cos, K_norm


def _run(x_spatial, attn, W, b, trace=False):
    nc = _build_kernel()
    in_maps = _make_in_maps(x_spatial, attn, W, b)
    res = run_bass_kernel_spmd(nc, in_maps, list(range(N_CORES)), trace=trace)
    return _assemble(res.results), res


def kernel(x_spatial, attn, W, b):
    (gate, cos, K_norm), _ = _run(x_spatial, attn, W, b, trace=False)
    return gate, cos, K_norm


# revision 35
# speedup vs baseline: 1.0539x; 1.0539x over previous
"""AngularPhysicsGate distributed Trainium2 kernel (8 NeuronCores).

reference (per batch b):
    x_unit = x / clip(||x||_row, eps)
    K      = x_unit @ x_unit^T                      [L, L]
    K_norm = (K - mean(K)) / clip(std(K, ddof=1), eps)
    A      = mean_h attn[b]                         [L, L]
    cos    = sum_m A * K_norm                       [L]
    gate   = sigmoid(W * cos + b)

Sharding: 8 cores = 2 batches x 4 row-chunks of 512 rows.  Each core:
  - loads the full x[b] (rolled so its own 512 rows sit at rows 0..511,
    keeping the SPMD program identical across cores), normalizes rows,
    transposes to [d, L] bf16 via the PE array,
  - computes its K rows with PE matmuls (bf16 operands, fp32 PSUM accum),
  - accumulates sum / sum-of-squares of K, AllReduces them across the 4
    cores of its batch to get the global mean/std,
  - streams its attn rows, head-sums on DVE, and folds cos_align through
    the raw-K linear decomposition:
        cos = inv_std * (sum_m A*K - mean * sum_m A)
  - writes K_norm rows, cos, gate.
Host side un-rolls the column rotation and concatenates shards.
"""

import numpy as np

import concourse.bass as bass
import concourse.bacc as bacc
import concourse.tile as tile
import concourse.mybir as mybir
from concourse import bass_isa
from concourse.masks import make_identity
from concourse.bass_utils import run_bass_kernel_spmd

F32 = mybir.dt.float32
F32R = mybir.dt.float32r
BF16 = mybir.dt.bfloat16
ALU = mybir.AluOpType
ACTF = mybir.ActivationFunctionType

B, L, D, H = 2, 2048, 2048, 8
P = 128
RC = L // 4                 # 512 rows per core
NSLAB = L // P              # 16 x-slabs
NK = D // P                 # 16 contraction tiles
NR = RC // P                # 4 row subtiles per core
NC_FREE = 512               # matmul moving free dim / PSUM bank
NCG = L // NC_FREE          # 4 column groups
N_CORES = 8
EPS = 1e-6
NTOT = float(L) * float(L)

REPLICA_GROUPS = [[0, 1, 2, 3], [4, 5, 6, 7]]

_NC_CACHE = None


def _build_kernel():
    global _NC_CACHE
    if _NC_CACHE is not None:
        return _NC_CACHE

    nc = bacc.Bacc(
        "TRN2",
        target_bir_lowering=False,
        debug=False,
        num_devices=N_CORES,
    )

    x_ext = nc.dram_tensor("x", [L, D], F32, kind="ExternalInput")
    at_ext = nc.dram_tensor("attn", [H, RC, L], F32, kind="ExternalInput")
    wb_ext = nc.dram_tensor("wb", [1, 2], F32, kind="ExternalInput")
    kn_ext = nc.dram_tensor("k_norm", [RC, L], F32, kind="ExternalOutput")
    cos_ext = nc.dram_tensor("cos", [P, NR], F32, kind="ExternalOutput")
    gate_ext = nc.dram_tensor("gate", [P, NR], F32, kind="ExternalOutput")

    with tile.TileContext(nc) as tc:
        _body(nc, tc, x_ext, at_ext, wb_ext, kn_ext, cos_ext, gate_ext)

    nc.compile()
    _NC_CACHE = nc
    return nc


def _body(nc, tc, x_ext, at_ext, wb_ext, kn_ext, cos_ext, gate_ext):
    with (
        tc.tile_pool(name="consts", bufs=1) as consts,
        tc.tile_pool(name="xin", bufs=3) as xin_pool,
        tc.tile_pool(name="xu", bufs=2) as xu_pool,
        tc.tile_pool(name="xut", bufs=1) as xut_pool,
        tc.tile_pool(name="ksb", bufs=1) as ksb_pool,
        tc.tile_pool(name="attn", bufs=5) as at_pool,
        tc.tile_pool(name="acc", bufs=2) as acc_pool,
        tc.tile_pool(name="scr", bufs=2) as scr_pool,
        tc.tile_pool(name="small", bufs=1) as small,
        tc.tile_pool(name="psmm", bufs=4, space="PSUM") as psmm_pool,
        tc.tile_pool(name="pstr", bufs=3, space="PSUM") as pstr_pool,
        tc.tile_pool(name="dram", bufs=1, space="DRAM") as dram_pool,
    ):
        ident = consts.tile([P, P], BF16)
        make_identity(nc, ident)

        # W / bias, broadcast to all partitions
        wb_sb = small.tile([1, 2], F32)
        nc.sync.dma_start(wb_sb[:], wb_ext[:, :])
        wb_bc = small.tile([P, 2], F32)
        nc.gpsimd.partition_broadcast(wb_bc[:], wb_sb[:], channels=P)

        xuT = xut_pool.tile([P, NK, L], BF16)     # x_unit^T, [d-part, ktile, row]
        K_sb = ksb_pool.tile([P, NR, L], BF16)    # this core's K rows (cosine)

        ss_all = small.tile([P, NSLAB], F32)      # row sum-of-squares per slab
        inv_all = small.tile([P, NSLAB], F32)     # 1/max(sqrt(ss), eps)

        # ---- Stage 1: load x (scalar-engine queues), normalize rows,
        # cast bf16, PE-transpose to xuT ----
        last_x_dma = None
        for s in range(NSLAB):
            xs = xin_pool.tile([P, D], F32, tag="xs")
            last_x_dma = nc.sync.dma_start(xs[:], x_ext[s * P:(s + 1) * P, :])
            sq = scr_pool.tile([P, D], BF16, tag="sq")
            nc.scalar.activation(
                sq[:], xs[:], ACTF.Square, accum_out=ss_all[:, s:s + 1]
            )
            ssl = slice(s, s + 1)
            nc.scalar.activation(inv_all[:, ssl], ss_all[:, ssl], ACTF.Sqrt)
            nc.vector.tensor_scalar_max(inv_all[:, ssl], inv_all[:, ssl], EPS)
            nc.vector.reciprocal(inv_all[:, ssl], inv_all[:, ssl])
            xu = xu_pool.tile([P, D], BF16, tag="xu")
            nc.vector.tensor_scalar_mul(xu[:], xs[:], inv_all[:, ssl])
            for tg in range(4):
                ps = pstr_pool.tile([P, 4, P], BF16, tag="pstr")
                for t4 in range(4):
                    t = tg * 4 + t4
                    nc.tensor.transpose(
                        ps[:, t4], xu[:, t * P:(t + 1) * P], ident
                    )
                dst = xuT[:, tg * 4:(tg + 1) * 4, s * P:(s + 1) * P]
                if tg % 2 == 0:
                    nc.scalar.activation(dst, ps[:, :, :], ACTF.Copy)
                else:
                    nc.vector.tensor_copy(dst, ps[:, :, :])

        # ---- Stage 2: K = xu @ xu^T; drains carry the stats for free ----
        sum_slots = small.tile([P, NR * NCG], F32)
        ssq_slots = small.tile([P, NR * NCG], F32)
        for c in range(NCG):
            for r in range(NR):
                pk = psmm_pool.tile([P, NC_FREE], F32, tag="pk")
                for k in range(NK):
                    nc.tensor.matmul(
                        pk[:],
                        xuT[:, k, r * P:(r + 1) * P],
                        xuT[:, k, c * NC_FREE:(c + 1) * NC_FREE],
                        start=(k == 0),
                        stop=(k == NK - 1),
                    )
                idx = r * NCG + c
                csl = slice(c * NC_FREE, (c + 1) * NC_FREE)
                nc.scalar.activation(
                    K_sb[:, r, csl], pk[:], ACTF.Identity,
                    accum_out=sum_slots[:, idx:idx + 1],
                )
                sq2 = scr_pool.tile([P, NC_FREE], BF16, tag="sq2")
                nc.scalar.activation(
                    sq2[:], pk[:], ACTF.Square,
                    accum_out=ssq_slots[:, idx:idx + 1],
                )

        # ---- Stage 2b: global stats via AllReduce over the batch group ----
        stats2 = small.tile([P, 2], F32)
        nc.vector.tensor_reduce(
            stats2[:, 0:1], sum_slots[:], axis=mybir.AxisListType.X, op=ALU.add
        )
        nc.vector.tensor_reduce(
            stats2[:, 1:2], ssq_slots[:], axis=mybir.AxisListType.X, op=ALU.add
        )
        statsr = small.tile([P, 2], F32)
        nc.gpsimd.partition_all_reduce(
            statsr[:], stats2[:], channels=P, reduce_op=bass_isa.ReduceOp.add
        )
        stats_in = dram_pool.tile([P, 2], F32)
        stats_out = dram_pool.tile([P, 2], F32)
        nc.gpsimd.dma_start(stats_in[:], statsr[:])
        nc.gpsimd.collective_compute(
            "AllReduce",
            ALU.add,
            replica_groups=REPLICA_GROUPS,
            ins=[stats_in.opt()],
            outs=[stats_out.opt()],
        )
        statsg = small.tile([P, 2], F32)
        nc.gpsimd.dma_start(statsg[:], stats_out[:])

        # derived scalars, computed redundantly on every partition [P,1]
        mean_t = small.tile([P, 1], F32)
        nc.vector.tensor_scalar_mul(mean_t[:], statsg[:, 0:1], 1.0 / NTOT)
        t2 = small.tile([P, 1], F32)                              # S*mean - SS
        nc.vector.scalar_tensor_tensor(
            t2[:], statsg[:, 0:1], mean_t[:, 0:1], statsg[:, 1:2],
            op0=ALU.mult, op1=ALU.subtract,
        )
        std_t = small.tile([P, 1], F32)                           # sqrt(-t2/(N-1))
        nc.scalar.activation(std_t[:], t2[:], ACTF.Sqrt,
                             scale=-1.0 / (NTOT - 1.0))
        nc.vector.tensor_scalar_max(std_t[:], std_t[:], EPS)
        iv_t = small.tile([P, 1], F32)
        nc.vector.reciprocal(iv_t[:], std_t[:])
        s1_t = small.tile([P, 1], F32)                            # inv_std/H
        nc.vector.tensor_scalar_mul(s1_t[:], iv_t[:], 1.0 / H)
        s2_t = small.tile([P, 1], F32)                            # mean*inv_std/H
        nc.vector.tensor_mul(s2_t[:], mean_t[:], s1_t[:])
        nb_t = small.tile([P, 1], F32)                            # -mean*inv_std
        nc.vector.tensor_scalar_mul(nb_t[:], s2_t[:], -float(H))

        # ---- Stage 4: K_norm = K * inv_std - mean*inv_std, DMA out ----
        for r in range(NR):
            kn = xin_pool.tile([P, L], F32, tag="xs", name=f"kn_{r}")
            nc.scalar.activation(
                kn[:], K_sb[:, r, :], ACTF.Identity,
                bias=nb_t[:, 0:1], scale=iv_t[:, 0:1],
            )
            nc.gpsimd.dma_start(kn_ext[r * P:(r + 1) * P, :], kn[:])

        # ---- Stage 3: attn head-sum (DVE) + dot with cosine K ----
        dot_slots = small.tile([P, NR], F32)
        row_slots = small.tile([P, NR], F32)
        cos_sb = small.tile([P, NR], F32)
        gate_sb = small.tile([P, NR], F32)
        tmp = small.tile([P, NR], F32)
        for r in range(NR):
            a_acc = acc_pool.tile([P, L], F32, tag="acc")
            at0 = at_pool.tile([P, L], F32, tag="at")
            nc.sync.dma_start(at0[:], at_ext[0, r * P:(r + 1) * P, :])
            at1 = at_pool.tile([P, L], F32, tag="at")
            nc.sync.dma_start(at1[:], at_ext[1, r * P:(r + 1) * P, :])
            nc.vector.tensor_add(a_acc[:], at0[:], at1[:])
            for h in range(2, H):
                at = at_pool.tile([P, L], F32, tag="at")
                nc.sync.dma_start(at[:], at_ext[h, r * P:(r + 1) * P, :])
                if h < H - 1:
                    nc.vector.tensor_add(a_acc[:], a_acc[:], at[:])
                else:
                    # last head add also produces sum_m A (times H)
                    nc.vector.scalar_tensor_tensor(
                        a_acc[:], at[:], 1.0, a_acc[:],
                        op0=ALU.mult, op1=ALU.add,
                        accum_out=row_slots[:, r:r + 1],
                    )
            junk = scr_pool.tile([P, L], BF16, tag="sq")
            nc.vector.scalar_tensor_tensor(
                junk[:], a_acc[:], 1.0, K_sb[:, r, :],
                op0=ALU.mult, op1=ALU.mult,
                accum_out=dot_slots[:, r:r + 1],
            )
            # cos = s1 * dot - s2 * rowsum ; gate = sigmoid(W*cos + b)
            rsl = slice(r, r + 1)
            nc.vector.tensor_scalar_mul(tmp[:, rsl], row_slots[:, rsl],
                                        s2_t[:, 0:1])
            nc.vector.scalar_tensor_tensor(
                cos_sb[:, rsl], dot_slots[:, rsl], s1_t[:, 0:1], tmp[:, rsl],
                op0=ALU.mult, op1=ALU.subtract,
            )
            nc.scalar.activation(
                gate_sb[:, rsl], cos_sb[:, rsl], ACTF.Sigmoid,
                bias=wb_bc[:, 1:2], scale=wb_bc[:, 0:1],
            )
        nc.sync.dma_start(cos_ext[:, :], cos_sb[:])
        nc.sync.dma_start(gate_ext[:, :], gate_sb[:])


def _make_in_maps(x_spatial, attn, W, b):
    x_spatial = np.ascontiguousarray(x_spatial, dtype=np.float32)
    attn = np.ascontiguousarray(attn, dtype=np.float32)
    wb = np.array([[np.float32(np.ravel(W)[0]), np.float32(np.ravel(b)[0])]],
                  dtype=np.float32)
    in_maps = []
    for core in range(N_CORES):
        bb, rc = divmod(core, 4)
        r0 = rc * RC
        x_roll = np.roll(x_spatial[bb], -r0, axis=0)
        at_roll = np.roll(attn[bb, :, r0:r0 + RC, :], -r0, axis=2)
        in_maps.append({
            "x": np.ascontiguousarray(x_roll),
            "attn": np.ascontiguousarray(at_roll),
            "wb": wb,
        })
    return in_maps


def _assemble(results):
    gate = np.empty((B, L), np.float32)
    cos = np.empty((B, L), np.float32)
    K_norm = np.empty((B, L, L), np.float32)
    for core in range(N_CORES):
        bb, rc = divmod(core, 4)
        r0 = rc * RC
        out = results[core]
        K_norm[bb, r0:r0 + RC] = np.roll(out["k_norm"], r0, axis=1)
        cos[bb, r0:r0 + RC] = out["cos"].T.reshape(RC)
        gate[bb, r0:r0 + RC] = out["gate"].T.reshape(RC)
    return gate, cos, K_norm


def _run(x_spatial, attn, W, b, trace=False):
    nc = _build_kernel()
    in_maps = _make_in_maps(x_spatial, attn, W, b)
    res = run_bass_kernel_spmd(nc, in_maps, list(range(N_CORES)), trace=trace)
    return _assemble(res.results), res


def kernel(x_spatial, attn, W, b):
    (gate, cos, K_norm), _ = _run(x_spatial, attn, W, b, trace=False)
    return gate, cos, K_norm


# revision 36
# speedup vs baseline: 1.0924x; 1.0366x over previous
"""AngularPhysicsGate distributed Trainium2 kernel (8 NeuronCores).

reference (per batch b):
    x_unit = x / clip(||x||_row, eps)
    K      = x_unit @ x_unit^T                      [L, L]
    K_norm = (K - mean(K)) / clip(std(K, ddof=1), eps)
    A      = mean_h attn[b]                         [L, L]
    cos    = sum_m A * K_norm                       [L]
    gate   = sigmoid(W * cos + b)

Sharding: 8 cores = 2 batches x 4 row-chunks of 512 rows.  Each core:
  - loads the full x[b] (rolled so its own 512 rows sit at rows 0..511,
    keeping the SPMD program identical across cores), normalizes rows,
    transposes to [d, L] bf16 via the PE array,
  - computes its K rows with PE matmuls (bf16 operands, fp32 PSUM accum),
  - accumulates sum / sum-of-squares of K, AllReduces them across the 4
    cores of its batch to get the global mean/std,
  - streams its attn rows, head-sums on DVE, and folds cos_align through
    the raw-K linear decomposition:
        cos = inv_std * (sum_m A*K - mean * sum_m A)
  - writes K_norm rows, cos, gate.
Host side un-rolls the column rotation and concatenates shards.
"""

import numpy as np

import concourse.bass as bass
import concourse.bacc as bacc
import concourse.tile as tile
import concourse.mybir as mybir
from concourse import bass_isa
from concourse.masks import make_identity
from concourse.bass_utils import run_bass_kernel_spmd

F32 = mybir.dt.float32
F32R = mybir.dt.float32r
BF16 = mybir.dt.bfloat16
ALU = mybir.AluOpType
ACTF = mybir.ActivationFunctionType

B, L, D, H = 2, 2048, 2048, 8
P = 128
RC = L // 4                 # 512 rows per core
NSLAB = L // P              # 16 x-slabs
NK = D // P                 # 16 contraction tiles
NR = RC // P                # 4 row subtiles per core
NC_FREE = 512               # matmul moving free dim / PSUM bank
NCG = L // NC_FREE          # 4 column groups
N_CORES = 8
EPS = 1e-6
NTOT = float(L) * float(L)

REPLICA_GROUPS = [[0, 1, 2, 3], [4, 5, 6, 7]]

_NC_CACHE = None


def _build_kernel():
    global _NC_CACHE
    if _NC_CACHE is not None:
        return _NC_CACHE

    nc = bacc.Bacc(
        "TRN2",
        target_bir_lowering=False,
        debug=False,
        num_devices=N_CORES,
    )

    x_ext = nc.dram_tensor("x", [L, D], F32, kind="ExternalInput")
    at_ext = nc.dram_tensor("attn", [H, RC, L], F32, kind="ExternalInput")
    wb_ext = nc.dram_tensor("wb", [1, 2], F32, kind="ExternalInput")
    kn_ext = nc.dram_tensor("k_norm", [RC, L], F32, kind="ExternalOutput")
    cos_ext = nc.dram_tensor("cos", [P, NR], F32, kind="ExternalOutput")
    gate_ext = nc.dram_tensor("gate", [P, NR], F32, kind="ExternalOutput")

    with tile.TileContext(nc) as tc:
        _body(nc, tc, x_ext, at_ext, wb_ext, kn_ext, cos_ext, gate_ext)

    nc.compile()
    _NC_CACHE = nc
    return nc


def _body(nc, tc, x_ext, at_ext, wb_ext, kn_ext, cos_ext, gate_ext):
    with (
        tc.tile_pool(name="consts", bufs=1) as consts,
        tc.tile_pool(name="xin", bufs=3) as xin_pool,
        tc.tile_pool(name="xu", bufs=2) as xu_pool,
        tc.tile_pool(name="xut", bufs=1) as xut_pool,
        tc.tile_pool(name="ksb", bufs=1) as ksb_pool,
        tc.tile_pool(name="attn", bufs=5) as at_pool,
        tc.tile_pool(name="acc", bufs=2) as acc_pool,
        tc.tile_pool(name="scr", bufs=2) as scr_pool,
        tc.tile_pool(name="small", bufs=1) as small,
        tc.tile_pool(name="psmm", bufs=4, space="PSUM") as psmm_pool,
        tc.tile_pool(name="pstr", bufs=3, space="PSUM") as pstr_pool,
        tc.tile_pool(name="dram", bufs=1, space="DRAM") as dram_pool,
    ):
        ident = consts.tile([P, P], BF16)
        make_identity(nc, ident)

        # W / bias, broadcast to all partitions
        wb_sb = small.tile([1, 2], F32)
        nc.sync.dma_start(wb_sb[:], wb_ext[:, :])
        wb_bc = small.tile([P, 2], F32)
        nc.gpsimd.partition_broadcast(wb_bc[:], wb_sb[:], channels=P)

        xuT = xut_pool.tile([P, NK, L], BF16)     # x_unit^T, [d-part, ktile, row]
        K_sb = ksb_pool.tile([P, NR, L], BF16)    # this core's K rows (cosine)

        ss_all = small.tile([P, NSLAB], F32)      # row sum-of-squares per slab
        inv_all = small.tile([P, NSLAB], F32)     # 1/max(sqrt(ss), eps)

        # ---- Stage 1: load x (scalar-engine queues), normalize rows,
        # cast bf16, PE-transpose to xuT ----
        last_x_dma = None
        for s in range(NSLAB):
            xs = xin_pool.tile([P, D], F32, tag="xs")
            last_x_dma = nc.sync.dma_start(xs[:], x_ext[s * P:(s + 1) * P, :])
            sq = scr_pool.tile([P, D], BF16, tag="sq")
            nc.scalar.activation(
                sq[:], xs[:], ACTF.Square, accum_out=ss_all[:, s:s + 1]
            )
            ssl = slice(s, s + 1)
            nc.scalar.activation(inv_all[:, ssl], ss_all[:, ssl], ACTF.Sqrt)
            nc.vector.tensor_scalar_max(inv_all[:, ssl], inv_all[:, ssl], EPS)
            nc.vector.reciprocal(inv_all[:, ssl], inv_all[:, ssl])
            xu = xu_pool.tile([P, D], BF16, tag="xu")
            nc.vector.tensor_scalar_mul(xu[:], xs[:], inv_all[:, ssl])
            for tg in range(4):
                ps = pstr_pool.tile([P, 4, P], BF16, tag="pstr")
                for t4 in range(4):
                    t = tg * 4 + t4
                    nc.tensor.transpose(
                        ps[:, t4], xu[:, t * P:(t + 1) * P], ident
                    )
                dst = xuT[:, tg * 4:(tg + 1) * 4, s * P:(s + 1) * P]
                if tg % 2 == 0:
                    nc.scalar.activation(dst, ps[:, :, :], ACTF.Copy)
                else:
                    nc.vector.tensor_copy(dst, ps[:, :, :])

        # ---- Stage 2: K = xu @ xu^T; drains carry the stats for free ----
        sum_slots = small.tile([P, NR * NCG], F32)
        ssq_slots = small.tile([P, NR * NCG], F32)
        for c in range(NCG):
            for r in range(NR):
                pk = psmm_pool.tile([P, NC_FREE], F32, tag="pk")
                for k in range(NK):
                    nc.tensor.matmul(
                        pk[:],
                        xuT[:, k, r * P:(r + 1) * P],
                        xuT[:, k, c * NC_FREE:(c + 1) * NC_FREE],
                        start=(k == 0),
                        stop=(k == NK - 1),
                    )
                idx = r * NCG + c
                csl = slice(c * NC_FREE, (c + 1) * NC_FREE)
                nc.scalar.activation(
                    K_sb[:, r, csl], pk[:], ACTF.Identity,
                    accum_out=sum_slots[:, idx:idx + 1],
                )
                sq2 = scr_pool.tile([P, NC_FREE], BF16, tag="sq2")
                nc.scalar.activation(
                    sq2[:], pk[:], ACTF.Square,
                    accum_out=ssq_slots[:, idx:idx + 1],
                )

        # ---- Stage 2b: global stats via AllReduce over the batch group ----
        stats2 = small.tile([P, 2], F32)
        nc.vector.tensor_reduce(
            stats2[:, 0:1], sum_slots[:], axis=mybir.AxisListType.X, op=ALU.add
        )
        nc.vector.tensor_reduce(
            stats2[:, 1:2], ssq_slots[:], axis=mybir.AxisListType.X, op=ALU.add
        )
        statsr = small.tile([P, 2], F32)
        nc.gpsimd.partition_all_reduce(
            statsr[:], stats2[:], channels=P, reduce_op=bass_isa.ReduceOp.add
        )
        stats_in = dram_pool.tile([P, 2], F32)
        stats_out = dram_pool.tile([P, 2], F32)
        nc.gpsimd.dma_start(stats_in[:], statsr[:])
        nc.gpsimd.collective_compute(
            "AllReduce",
            ALU.add,
            replica_groups=REPLICA_GROUPS,
            ins=[stats_in.opt()],
            outs=[stats_out.opt()],
        )
        statsg = small.tile([P, 2], F32)
        nc.gpsimd.dma_start(statsg[:], stats_out[:])

        # derived scalars, computed redundantly on every partition [P,1]
        mean_t = small.tile([P, 1], F32)
        nc.vector.tensor_scalar_mul(mean_t[:], statsg[:, 0:1], 1.0 / NTOT)
        t2 = small.tile([P, 1], F32)                              # S*mean - SS
        nc.vector.scalar_tensor_tensor(
            t2[:], statsg[:, 0:1], mean_t[:, 0:1], statsg[:, 1:2],
            op0=ALU.mult, op1=ALU.subtract,
        )
        std_t = small.tile([P, 1], F32)                           # sqrt(-t2/(N-1))
        nc.scalar.activation(std_t[:], t2[:], ACTF.Sqrt,
                             scale=-1.0 / (NTOT - 1.0))
        nc.vector.tensor_scalar_max(std_t[:], std_t[:], EPS)
        iv_t = small.tile([P, 1], F32)
        nc.vector.reciprocal(iv_t[:], std_t[:])
        s1_t = small.tile([P, 1], F32)                            # inv_std/H
        nc.vector.tensor_scalar_mul(s1_t[:], iv_t[:], 1.0 / H)
        s2_t = small.tile([P, 1], F32)                            # mean*inv_std/H
        nc.vector.tensor_mul(s2_t[:], mean_t[:], s1_t[:])
        nb_t = small.tile([P, 1], F32)                            # -mean*inv_std
        nc.vector.tensor_scalar_mul(nb_t[:], s2_t[:], -float(H))

        # ---- Stage 4: K_norm = K * inv_std - mean*inv_std, DMA out ----
        for r in range(NR):
            kn = xin_pool.tile([P, L], F32, tag="xs", name=f"kn_{r}")
            nc.scalar.activation(
                kn[:], K_sb[:, r, :], ACTF.Identity,
                bias=nb_t[:, 0:1], scale=iv_t[:, 0:1],
            )
            nc.gpsimd.dma_start(kn_ext[r * P:(r + 1) * P, :], kn[:])

        # ---- Stage 3: attn head-sum (DVE) + dot with cosine K ----
        dot_slots = small.tile([P, NR], F32)
        row_slots = small.tile([P, NR], F32)
        cos_sb = small.tile([P, NR], F32)
        gate_sb = small.tile([P, NR], F32)
        tmp = small.tile([P, NR], F32)
        for r in range(NR):
            a_acc = acc_pool.tile([P, L], F32, tag="acc")
            at0 = at_pool.tile([P, L], F32, tag="at")
            nc.sync.dma_start(at0[:], at_ext[0, r * P:(r + 1) * P, :])
            at1 = at_pool.tile([P, L], F32, tag="at")
            nc.sync.dma_start(at1[:], at_ext[1, r * P:(r + 1) * P, :])
            nc.vector.tensor_add(a_acc[:], at0[:], at1[:])
            for h in range(2, H):
                at = at_pool.tile([P, L], F32, tag="at")
                nc.sync.dma_start(at[:], at_ext[h, r * P:(r + 1) * P, :])
                if h < H - 1:
                    nc.vector.tensor_add(a_acc[:], a_acc[:], at[:])
                else:
                    # last head add also produces sum_m A (times H)
                    nc.vector.scalar_tensor_tensor(
                        a_acc[:], at[:], 1.0, a_acc[:],
                        op0=ALU.mult, op1=ALU.add,
                        accum_out=row_slots[:, r:r + 1],
                    )
            junk = scr_pool.tile([P, L], BF16, tag="sq")
            nc.vector.scalar_tensor_tensor(
                junk[:], a_acc[:], 1.0, K_sb[:, r, :],
                op0=ALU.mult, op1=ALU.mult,
                accum_out=dot_slots[:, r:r + 1],
            )
        # cos = s1 * dot - s2 * rowsum ; gate = sigmoid(W*cos + b)
        # batched AFTER the loop: these wait on the collective, and putting
        # them inside the loop head-of-line blocks the in-order DVE queue
        # (stalling the adds and, через the at-slot releases, the attn DMA).
        nc.vector.tensor_scalar_mul(tmp[:], row_slots[:], s2_t[:, 0:1])
        nc.vector.scalar_tensor_tensor(
            cos_sb[:], dot_slots[:], s1_t[:, 0:1], tmp[:],
            op0=ALU.mult, op1=ALU.subtract,
        )
        nc.scalar.activation(
            gate_sb[:], cos_sb[:], ACTF.Sigmoid,
            bias=wb_bc[:, 1:2], scale=wb_bc[:, 0:1],
        )
        nc.sync.dma_start(cos_ext[:, :], cos_sb[:])
        nc.sync.dma_start(gate_ext[:, :], gate_sb[:])


def _make_in_maps(x_spatial, attn, W, b):
    x_spatial = np.ascontiguousarray(x_spatial, dtype=np.float32)
    attn = np.ascontiguousarray(attn, dtype=np.float32)
    wb = np.array([[np.float32(np.ravel(W)[0]), np.float32(np.ravel(b)[0])]],
                  dtype=np.float32)
    in_maps = []
    for core in range(N_CORES):
        bb, rc = divmod(core, 4)
        r0 = rc * RC
        x_roll = np.roll(x_spatial[bb], -r0, axis=0)
        at_roll = np.roll(attn[bb, :, r0:r0 + RC, :], -r0, axis=2)
        in_maps.append({
            "x": np.ascontiguousarray(x_roll),
            "attn": np.ascontiguousarray(at_roll),
            "wb": wb,
        })
    return in_maps


def _assemble(results):
    gate = np.empty((B, L), np.float32)
    cos = np.empty((B, L), np.float32)
    K_norm = np.empty((B, L, L), np.float32)
    for core in range(N_CORES):
        bb, rc = divmod(core, 4)
        r0 = rc * RC
        out = results[core]
        K_norm[bb, r0:r0 + RC] = np.roll(out["k_norm"], r0, axis=1)
        cos[bb, r0:r0 + RC] = out["cos"].T.reshape(RC)
        gate[bb, r0:r0 + RC] = out["gate"].T.reshape(RC)
    return gate, cos, K_norm


def _run(x_spatial, attn, W, b, trace=False):
    nc = _build_kernel()
    in_maps = _make_in_maps(x_spatial, attn, W, b)
    res = run_bass_kernel_spmd(nc, in_maps, list(range(N_CORES)), trace=trace)
    return _assemble(res.results), res


def kernel(x_spatial, attn, W, b):
    (gate, cos, K_norm), _ = _run(x_spatial, attn, W, b, trace=False)
    return gate, cos, K_norm
